# revision 40
# baseline (speedup 1.0000x reference)
"""GCNEvaluator Trainium2 kernel: 8-core SPMD, dst-partitioned GNN.

Sharding: nodes split into 8 contiguous ranges (N/8 per core); edges bucketed
by (dst core, dst tile of 128 nodes, src range of 32768 nodes) on the host,
padded to a shared (SPMD-uniform) chunk structure of 128-edge chunks.

Per core, channel-on-partition layout ([64ch, nodes] in SBUF):
  P1: x_ = Wi @ x.T + bi, h = x_              (XH = [x_ ; h], SBUF-resident)
  P2: ew = relu(relu(ea@W1t)@W2t), pair-packed: two chunks per [128, 512]
      block via block-diagonal W1/W2 (full 128-partition MLP); PE-transposed
      to edge-major [128e, 64c] tiles and stored to DRAM; degrees
      deg = sum_e ew (+1 self loop) accumulated in the same pass via matmul
      against a one-hot dst matrix A; dinv = 1/sqrt(deg+1) (bf16, DGd tile).
  layer l: g = dinv * relu(wc[l]*h) in bulk [64, NPD] instrs (DGg, bf16);
      PE-transpose g per 128-node tile into a staging buffer, 2 DMAs to
      gloc, AllGather the full bf16 gather table; per group of 4 dst tiles
      (= one 512-f32 PSUM bank): dma_gather source pair-rows (one call per
      src range, spread over 4 SWDGE queues, per-range buffers), vals =
      ew (.) g_src multiplied in place into the gather buffers,
      matmul-accumulate vals^T @ A into the group PSUM; then bulk
      h_conv = dinv*(psum + g) + b_conv and h = Wl' @ [x_ ; h_conv] where
      Wl' has the residual [I ; 0] folded in host-side.
  readout: out = Wo @ [x_ ; relu(h)].

Gathers ride 4 SWDGE queues (queue = global gather counter % 4) - a DMASW
semaphore may only be updated from one queue, so sim builds use one_queue.

Self-contained: imports only concourse (staged on the machine) + numpy.
"""

import os
import sys

for _p in ("/opt/trn_rl_repo", os.path.expanduser("~/.axon_site/_ro/trn_rl_repo")):
    if os.path.isdir(_p) and _p not in sys.path:
        sys.path.insert(0, _p)

import numpy as np
import ml_dtypes

import concourse.bass as bass
import concourse.bacc as bacc
import concourse.mybir as mybir
import concourse.tile as tile
from concourse.bass_utils import run_bass_kernel_spmd
from concourse.masks import make_identity

bf16 = mybir.dt.bfloat16
f32 = mybir.dt.float32
nbf16 = ml_dtypes.bfloat16

NDEV = 8
GRP = 4  # dst tiles per gather group (= one 512-f32 PSUM bank)
RANGE = 32768  # max rows addressable by int16 gather indices


class Prep:
    pass


def preprocess(edge_index, edge_attr, N):
    E = edge_index.shape[1]
    NPD = N // NDEV
    T = (NPD + 127) // 128
    # buckets: (src range of 2*RANGE nodes) x (src parity); gather fetches
    # bf16 pair-rows (256B) so idx = src>>1 fits int16 within a range
    NR2 = (N + 2 * RANGE - 1) // (2 * RANGE)
    NR = NR2 * 2
    NG = (T + GRP - 1) // GRP

    src = np.asarray(edge_index[0], dtype=np.int64)
    dst = np.asarray(edge_index[1], dtype=np.int64)
    ea = np.asarray(edge_attr, dtype=np.float32)

    r = (src // (2 * RANGE)) * 2 + (src & 1)
    dev = dst // NPD
    ldst = dst - dev * NPD
    t = ldst >> 7
    drel = ldst & 127

    key = (dev * T + t) * NR + r
    order = np.argsort(key, kind="stable")
    counts = np.bincount(key, minlength=NDEV * T * NR).reshape(NDEV, T, NR)

    K = (counts.max(axis=0) + 127) // 128  # [T, NR] chunks per bucket (shared)
    CT = int(K.sum())
    SLOTS = CT * 128

    chunk_base = np.zeros((T, NR), dtype=np.int64)
    cc = 0
    groups = []
    for g in range(NG):
        tiles = list(range(g * GRP, min((g + 1) * GRP, T)))
        ginfo = {"tiles": tiles, "chunk0": cc, "calls": []}
        for rr in range(NR):
            c0 = cc
            for tt in tiles:
                chunk_base[tt, rr] = cc
                cc += int(K[tt, rr])
            ginfo["calls"].append((c0, cc - c0))
        ginfo["nchunks"] = cc - ginfo["chunk0"]
        groups.append(ginfo)
    assert cc == CT

    tile_chunks = [
        [int(chunk_base[tt, rr]) + k for rr in range(NR) for k in range(int(K[tt, rr]))]
        for tt in range(T)
    ]

    # pair-packed edge attrs for the 128-partition P2 MLP: pair q of a
    # group packs chunks (2q, 2q+1) as partition rows 0:8 / 8:16
    pair_base = []
    ptot = 0
    for g in groups:
        pair_base.append(ptot)
        ptot += (g["nchunks"] + 1) // 2
    eaT = np.zeros((NDEV, 8, SLOTS), dtype=nbf16)
    dstf = np.zeros((NDEV, 128, CT), dtype=nbf16)
    idx_rel = np.zeros((NDEV, SLOTS), dtype=np.int16)

    s_src = src[order]
    s_r = r[order]
    s_drel = drel[order]
    s_key = key[order]
    s_ea = ea[order]

    bstart = np.zeros(NDEV * T * NR + 1, dtype=np.int64)
    np.cumsum(np.bincount(s_key, minlength=NDEV * T * NR), out=bstart[1:])
    slot_of_bucket = (chunk_base * 128).astype(np.int64)

    for d in range(NDEV):
        for tt in range(T):
            for rr in range(NR):
                b = (d * T + tt) * NR + rr
                e0, e1 = int(bstart[b]), int(bstart[b + 1])
                n = e1 - e0
                if n == 0:
                    continue
                s0 = int(slot_of_bucket[tt, rr])
                sl = np.arange(s0, s0 + n)
                eaT[d][:, sl] = s_ea[e0:e1].T
                idx_rel[d][sl] = (
                    (s_src[e0:e1] - (rr // 2) * 2 * RANGE) >> 1
                ).astype(np.int16)
                dstf[d][sl % 128, sl // 128] = s_drel[e0:e1].astype(nbf16)

    idx16 = np.zeros((NDEV, 128, CT * 8), dtype=np.int16)
    for g in groups:
        for rr in range(NR):
            c0, nch = g["calls"][rr]
            if nch == 0:
                continue
            s0, s1 = c0 * 128, (c0 + nch) * 128
            colbase, ncols = c0 * 8, nch * 8
            for d in range(NDEV):
                seg = idx_rel[d][s0:s1].reshape(ncols, 16).T
                idx16[d][:, colbase : colbase + ncols] = np.tile(seg, (8, 1))

    eaT2 = np.zeros((NDEV, 16, ptot * 128), dtype=nbf16)
    for gi, g in enumerate(groups):
        c0, nch = g["chunk0"], g["nchunks"]
        npair = (nch + 1) // 2
        pb = pair_base[gi]
        for q in range(npair):
            cl = c0 + 2 * q
            sl_l = slice(cl * 128, (cl + 1) * 128)
            eaT2[:, 0:8, (pb + q) * 128 : (pb + q + 1) * 128] = eaT[:, :, sl_l]
            if 2 * q + 1 < nch:
                sl_r = slice((cl + 1) * 128, (cl + 2) * 128)
                eaT2[:, 8:16, (pb + q) * 128 : (pb + q + 1) * 128] =                     eaT[:, :, sl_r]

    p = Prep()
    p.N, p.E, p.NPD, p.T, p.NR, p.NG, p.CT = N, E, NPD, T, NR, NG, CT
    p.K, p.groups, p.tile_chunks = K, groups, tile_chunks
    p.eaT, p.dstf, p.idx16 = eaT2, dstf, idx16
    p.pair_base, p.PTOT = pair_base, ptot
    p.NPAIR = max((g["nchunks"] + 1) // 2 for g in groups)
    # pair-rows per bucket's source range
    p.bucket_rows = [
        (min(2 * RANGE, N - (rr // 2) * 2 * RANGE) + 1) // 2 for rr in range(NR)
    ]
    p.NCH = max(g["nchunks"] for g in groups)
    p.NCR = max(crn for g in groups for (_c0, crn) in g["calls"])
    # merged parity-pair calls: ranges (2k, 2k+1) share base + index space;
    # per-pair max sizes (pair 0 covers ~2x the sources of pair 1)
    p.NCR2S = [
        max(g["calls"][2 * rp][1] + g["calls"][2 * rp + 1][1] for g in groups)
        for rp in range(NR // 2)
    ]
    return p


def build_program(p, H, IN_DIM, NL, model_1core=False, layers=None, with_p2=True,
                  no_gather=False, no_aggmm=False, one_queue=False,
                  no_ab=False, no_vals=False, no_coll=False):
    layers = NL if layers is None else layers
    NPD, T, NR, CT, NCH = p.NPD, p.T, p.NR, p.CT, p.NCH
    nc = bacc.Bacc(
        "TRN2", target_bir_lowering=False, debug=False,
        num_devices=1 if model_1core else NDEV,
        num_swdge_queues=4,
    )

    ea_d = nc.dram_tensor(
        "eaT", [16, p.PTOT * 128], bf16, kind="ExternalInput"
    ).ap()
    dst_d = nc.dram_tensor("dstf", [128, CT], bf16, kind="ExternalInput").ap()
    idx_d = nc.dram_tensor(
        "idx16", [128, CT * 8], mybir.dt.int16, kind="ExternalInput"
    ).ap()
    xT_d = nc.dram_tensor("xT", [IN_DIM, NPD], f32, kind="ExternalInput").ap()
    w1t_d = nc.dram_tensor("w1t", [16, 2 * H], bf16, kind="ExternalInput").ap()
    w2t_d = nc.dram_tensor("w2t", [2 * H, 2 * H], bf16,
                           kind="ExternalInput").ap()
    wit_d = nc.dram_tensor("wit", [IN_DIM, H], f32, kind="ExternalInput").ap()
    bi_d = nc.dram_tensor("bi", [H, 1], f32, kind="ExternalInput").ap()
    wc_d = nc.dram_tensor("wconv", [H, NL], f32, kind="ExternalInput").ap()
    bc_d = nc.dram_tensor("bconv", [H, NL], f32, kind="ExternalInput").ap()
    wlt_d = nc.dram_tensor("wlt", [NL, 2 * H, H], f32, kind="ExternalInput").ap()
    wot_d = nc.dram_tensor("wot", [2 * H, 1], f32, kind="ExternalInput").ap()
    out_d = nc.dram_tensor("out", [1, NPD], f32, kind="ExternalOutput").ap()

    ew_d = nc.dram_tensor("ew_store", [128, CT * H], bf16).ap()
    gloc_d = nc.dram_tensor("g_loc", [NPD, H], bf16).ap()
    gfull_d = nc.dram_tensor("g_full", [p.N, H], bf16, addr_space="Shared").ap()

    rg = [list(range(NDEV))]
    AF = mybir.ActivationFunctionType
    _nidx_regs = {}
    # SWDGE queue per gather: global counter so the round-robin DMASW lane
    # (i % 8, in Pool program order) always pairs with queue (i % 4) — a
    # lane's semaphore may only ever be updated from one queue.
    _gq = [0]

    def nidx_reg(v):
        if v not in _nidx_regs:
            _nidx_regs[v] = nc.gpsimd.to_reg(v)
        return _nidx_regs[v]

    with tile.TileContext(nc) as tc:
        with (
            tc.tile_pool(name="const", bufs=1) as cp,
            tc.tile_pool(name="big", bufs=1) as bigp,
        ):
            w1t = cp.tile([16, 2 * H], bf16)
            nc.sync.dma_start(out=w1t[:], in_=w1t_d[:, :])
            w2t = cp.tile([2 * H, 2 * H], bf16)
            nc.sync.dma_start(out=w2t[:], in_=w2t_d[:, :])
            wit = cp.tile([IN_DIM, H], f32)
            nc.sync.dma_start(out=wit[:], in_=wit_d[:, :])
            bi = cp.tile([H, 1], f32)
            nc.sync.dma_start(out=bi[:], in_=bi_d[:, :])
            wc = cp.tile([H, NL], f32)
            nc.sync.dma_start(out=wc[:], in_=wc_d[:, :])
            bc = cp.tile([H, NL], f32)
            nc.sync.dma_start(out=bc[:], in_=bc_d[:, :])
            wlt = [
                cp.tile([2 * H, H], f32, name=f"wlt{l}", tag=f"wlt{l}")
                for l in range(NL)
            ]
            for l in range(layers):
                nc.sync.dma_start(out=wlt[l][:], in_=wlt_d[l, :, :])
            wot = cp.tile([2 * H, 1], f32)
            nc.sync.dma_start(out=wot[:], in_=wot_d[:, :])
            ident_b = cp.tile([H, H], bf16)
            make_identity(nc, ident_b[:])
            ident_b128 = cp.tile([128, 128], bf16)
            make_identity(nc, ident_b128[:])
            ident_f = cp.tile([H, H], f32)
            make_identity(nc, ident_f[:])
            iota_i = cp.tile([128, 128], mybir.dt.int32)
            nc.gpsimd.iota(iota_i[:], pattern=[[1, 128]], base=0, channel_multiplier=0)
            iota_b = cp.tile([128, 128], bf16)
            nc.vector.tensor_copy(iota_b[:], iota_i[:])
            dstf = bigp.tile([128, CT], bf16)
            nc.sync.dma_start(out=dstf[:], in_=dst_d[:, :])

            XH = bigp.tile([2 * H, NPD], f32)  # [x_ ; h]
            DGd = bigp.tile([H, NPD], bf16)  # dinv (bf16)
            DGg = bigp.tile([H, NPD], bf16)  # g (bf16: matches gather table)

            # ---------------- P1 ----------------
            with (
                tc.tile_pool(name="p1", bufs=3) as p1p,
                tc.tile_pool(name="p1ps", bufs=2, space="PSUM") as p1ps,
            ):
                for k0 in range(0, NPD, 512):
                    w = min(512, NPD - k0)
                    xk = p1p.tile([IN_DIM, 512], f32, tag="xk")
                    nc.sync.dma_start(out=xk[:, :w], in_=xT_d[:, k0 : k0 + w])
                    psx = p1ps.tile([H, 512], f32, tag="psx")
                    nc.tensor.matmul(
                        out=psx[:, :w], lhsT=wit[:], rhs=xk[:, :w], start=True, stop=True
                    )
                    nc.scalar.activation(
                        XH[0:H, k0 : k0 + w], psx[:, :w], AF.Identity, bias=bi[:]
                    )
                    nc.scalar.activation(
                        XH[H : 2 * H, k0 : k0 + w], psx[:, :w], AF.Identity, bias=bi[:]
                    )

            # ---------------- P2: ew + deg ----------------
            if not with_p2:
                nc.vector.memset(DGd[:, :], 1.0)
                nc.vector.memset(DGg[:, :], 1.0)
            with (
                tc.tile_pool(name="p2", bufs=2) as p2p,
                tc.tile_pool(name="p2s", bufs=3) as p2s,
                tc.tile_pool(name="p2A", bufs=1) as p2ap,
                tc.tile_pool(name="p2ps", bufs=1, space="PSUM") as p2ps,
                tc.tile_pool(name="degps", bufs=1, space="PSUM") as degps,
                tc.tile_pool(name="trps", bufs=2, space="PSUM") as trps,
            ):
                for gi, g in enumerate(p.groups if with_p2 else []):
                    nch = g["nchunks"]
                    if nch == 0:
                        continue
                    c0 = g["chunk0"]
                    npair = (nch + 1) // 2
                    pb = p.pair_base[gi]
                    eag = p2p.tile([16, p.NPAIR * 128], bf16, tag="eag")
                    nc.sync.dma_start(
                        out=eag[:, : npair * 128],
                        in_=ea_d[:, pb * 128 : (pb + npair) * 128],
                    )
                    Ag = p2ap.tile([128, NCH * 128], bf16, tag="Ag")
                    nc.vector.tensor_tensor(
                        out=Ag[:, : nch * 128].rearrange("p (c j) -> p c j", j=128),
                        in0=dstf[:, c0 : c0 + nch]
                        .unsqueeze(2)
                        .broadcast_to([128, nch, 128]),
                        in1=iota_b[:].unsqueeze(1).broadcast_to([128, nch, 128]),
                        op=mybir.AluOpType.is_equal,
                    )
                    dps, first, last, owner = {}, {}, {}, {}
                    for tt in g["tiles"]:
                        chs = p.tile_chunks[tt]
                        if chs:
                            dps[tt] = degps.tile([H, 128], f32, name=f"deg{tt % 4}", tag=f"deg{tt % 4}")
                            first[tt], last[tt] = chs[0], chs[-1]
                            for c in chs:
                                owner[c] = tt
                    for b0 in range(0, npair, 4):
                        nb = min(4, npair - b0)
                        bw = nb * 128
                        ps1 = p2ps.tile([2 * H, 512], f32, tag="ps1")
                        nc.tensor.matmul(
                            out=ps1[:, :bw], lhsT=w1t[:],
                            rhs=eag[:, b0 * 128 : b0 * 128 + bw],
                            start=True, stop=True,
                        )
                        s1 = p2s.tile([2 * H, 512], bf16, tag="s1")
                        nc.scalar.activation(s1[:, :bw], ps1[:, :bw], AF.Relu)
                        ps2 = p2ps.tile([2 * H, 512], f32, tag="ps2")
                        nc.tensor.matmul(
                            out=ps2[:, :bw], lhsT=w2t[:], rhs=s1[:, :bw],
                            start=True, stop=True,
                        )
                        s2 = p2s.tile([2 * H, 512], bf16, tag="s2")
                        nc.scalar.activation(s2[:, :bw], ps2[:, :bw], AF.Relu)
                        et = p2s.tile([128, 4 * 128], bf16, tag="et")
                        for j in range(nb):
                            q = b0 + j
                            pt = trps.tile([128, 128], bf16, tag="pt")
                            nc.tensor.transpose(
                                pt[:], s2[:, j * 128 : (j + 1) * 128],
                                ident_b128[:],
                            )
                            if j % 2 == 0:
                                nc.vector.tensor_copy(
                                    et[:, j * 128 : (j + 1) * 128], pt[:]
                                )
                            else:
                                nc.scalar.activation(
                                    et[:, j * 128 : (j + 1) * 128], pt[:],
                                    AF.Identity,
                                )
                            for half in (0, 1):
                                cc = c0 + 2 * q + half
                                if cc >= c0 + nch:
                                    continue
                                tt = owner[cc]
                                lc = cc - c0
                                nc.tensor.matmul(
                                    out=dps[tt][:],
                                    lhsT=et[:, j * 128 + half * H :
                                            j * 128 + half * H + H],
                                    rhs=Ag[:, lc * 128 : (lc + 1) * 128],
                                    start=(cc == first[tt]),
                                    stop=(cc == last[tt]),
                                )
                        real_w = min(2 * nb, nch - 2 * b0)
                        nc.sync.dma_start(
                            out=ew_d[:, (c0 + 2 * b0) * H :
                                     (c0 + 2 * b0 + real_w) * H],
                            in_=et[:, : real_w * H],
                        )
                    for tt in (g["tiles"] if with_p2 else []):
                        w = min(128, NPD - tt * 128)
                        if tt not in dps:
                            nc.vector.memset(DGd[:, tt * 128 : tt * 128 + w], 1.0)
                            continue
                        sq = p2s.tile([H, 128], f32, tag="sq")
                        nc.scalar.activation(sq[:], dps[tt][:], AF.Sqrt, bias=1.0)
                        with nc.allow_low_precision(reason="dinv stored bf16"):
                            nc.vector.reciprocal(
                                DGd[:, tt * 128 : tt * 128 + w], sq[:, :w]
                            )

            # ---------------- layers ----------------
            for l in range(layers):
                with (
                    tc.tile_pool(name=f"pa{l}", bufs=3) as pap,
                    tc.tile_pool(name=f"paps{l}", bufs=2, space="PSUM") as paps,
                ):
                    # pre-agg g = dinv * wc[l] * relu(h) in 512-col blocks
                    # aligned with post-agg groups, so layer l+1's blocks
                    # start as soon as layer l's matching group lands.
                    # relu(wc*h) == wc*relu(h) since w_conv > 0.
                    for b0 in range(0, NPD, 512):
                        bw = min(512, NPD - b0)
                        bsl = slice(b0, b0 + bw)
                        nc.scalar.activation(DGg[:, bsl], XH[H : 2 * H, bsl],
                                             AF.Relu, scale=wc[:, l : l + 1])
                        nc.vector.tensor_tensor(
                            out=DGg[:, bsl], in0=DGg[:, bsl], in1=DGd[:, bsl],
                            op=mybir.AluOpType.mult,
                        )
                    gsb = pap.tile([128, T * H], bf16, tag="gsb")
                    for tt in range(T):
                        w = min(128, NPD - tt * 128)
                        sl = slice(tt * 128, tt * 128 + w)
                        ptg = paps.tile([128, H], bf16, tag="ptg")
                        nc.tensor.transpose(ptg[:w, :], DGg[:, sl], ident_b[:])
                        nc.vector.tensor_copy(
                            gsb[:w, tt * H : (tt + 1) * H], ptg[:w, :]
                        )
                    TF = NPD // 128  # full 128-row tiles
                    nc.sync.dma_start(
                        out=gloc_d[0 : TF * 128, :].rearrange(
                            "(t q) h -> q t h", q=128
                        ),
                        in_=gsb[:, : TF * H].rearrange("p (t h) -> p t h", h=H),
                    )
                    if NPD > TF * 128:
                        nc.sync.dma_start(
                            out=gloc_d[TF * 128 : NPD, :],
                            in_=gsb[: NPD - TF * 128, TF * H : T * H],
                        )
                    if model_1core or no_coll:
                        # cost-model build: stand in for the AllGather with a
                        # local copy of this core's slice (collective adds
                        # ~25us/layer on HW, accounted separately)
                        nc.sync.dma_start(out=gfull_d[0:NPD, :], in_=gloc_d[:, :])
                    else:
                        nc.gpsimd.collective_compute(
                            "AllGather", mybir.AluOpType.bypass, replica_groups=rg,
                            ins=[gloc_d[:, :]], outs=[gfull_d[:, :]],
                        )

                with (
                    tc.tile_pool(name=f"pb{l}", bufs=2) as pbp,
                    tc.tile_pool(name=f"pbA{l}", bufs=1) as pbap,
                    tc.tile_pool(name=f"pbi{l}", bufs=3) as pbip,
                    tc.tile_pool(name=f"aggps{l}", bufs=2, space="PSUM") as aggps,
                    tc.tile_pool(name=f"mixps{l}", bufs=2, space="PSUM") as mixps,
                ):
                    Ab0 = None
                    if no_ab:
                        Ab0 = pbap.tile([128, NCH * 128], bf16, name="Ab0",
                                        tag="Ab")
                        nc.vector.memset(Ab0[:, :], 0.0)
                    for g in p.groups:
                        nch = g["nchunks"]
                        if nch == 0:
                            continue
                        c0 = g["chunk0"]
                        # per-range-pair gather buffers: own DMA semaphore
                        # per tile so calls ride separate SWDGE queues
                        gbufs = [
                            pbp.tile([128, p.NCR2S[rp] * 2 * H], bf16,
                                     name=f"gbuf{rp}", tag=f"gbuf{rp}")
                            for rp in range(NR // 2)
                        ]
                        idxg = pbip.tile([128, NCH * 8], mybir.dt.int16, tag="idxg")
                        nc.sync.dma_start(
                            out=idxg[:, : nch * 8],
                            in_=idx_d[:, c0 * 8 : (c0 + nch) * 8],
                        )
                        if no_gather:
                            for rp in range(NR // 2):
                                crn = (g["calls"][2 * rp][1]
                                       + g["calls"][2 * rp + 1][1])
                                if crn:
                                    nc.vector.memset(
                                        gbufs[rp][:, : crn * 2 * H], 0.0
                                    )
                        gpair = gfull_d[:, :].rearrange("(k two) h -> k (two h)", two=2)
                        for rp in (range(NR // 2) if not no_gather else []):
                            cr0, crn0 = g["calls"][2 * rp]
                            cr1, crn1 = g["calls"][2 * rp + 1]
                            assert cr1 == cr0 + crn0
                            crn = crn0 + crn1
                            if crn == 0:
                                continue
                            nidx = crn * 128
                            lc = cr0 - c0
                            rb = rp * RANGE
                            nc.gpsimd.dma_gather(
                                gbufs[rp][:, : crn * 2 * H].rearrange(
                                    "p (c j) -> p c j", j=2 * H
                                ),
                                gpair[rb : rb + p.bucket_rows[2 * rp], :],
                                idxg[:, lc * 8 : (lc + crn) * 8],
                                nidx, nidx_reg(nidx), 2 * H,
                                single_packet=False,
                                queue_num=0 if one_queue else _gq[0] % 4,
                            )
                            _gq[0] += 1
                        ewg = pbip.tile([128, NCH * H], bf16, tag="ewg")
                        nc.sync.dma_start(
                            out=ewg[:, : nch * H],
                            in_=ew_d[:, c0 * H : (c0 + nch) * H],
                        )
                        Ab = Ab0 if no_ab else pbap.tile(
                            [128, NCH * 128], bf16, name="Ab", tag="Ab")
                        if not no_ab:
                            nc.vector.tensor_tensor(
                                out=Ab[:, : nch * 128].rearrange(
                                    "p (c j) -> p c j", j=128
                                ),
                                in0=dstf[:, c0 : c0 + nch]
                                .unsqueeze(2)
                                .broadcast_to([128, nch, 128]),
                                in1=iota_b[:].unsqueeze(1).broadcast_to([128, nch, 128]),
                                op=mybir.AluOpType.is_equal,
                            )
                        # vals = ew (.) g_src, in place into the gather
                        # buffers (own half; other half unused garbage)
                        for rr in (range(NR) if not no_vals else []):
                            cr0, crn = g["calls"][rr]
                            if crn == 0:
                                continue
                            lc = cr0 - c0
                            half = (rr & 1) * H
                            base = g["calls"][(rr // 2) * 2][0]
                            off = cr0 - base
                            gsl = gbufs[rr // 2][
                                :, off * 2 * H : (off + crn) * 2 * H
                            ].rearrange(
                                "p (c j) -> p c j", j=2 * H
                            )[:, :, half : half + H]
                            nc.vector.tensor_tensor(
                                out=gsl, in0=gsl,
                                in1=ewg[:, lc * H : (lc + crn) * H].rearrange(
                                    "p (c j) -> p c j", j=H
                                ),
                                op=mybir.AluOpType.mult,
                            )

                        def val_slice(cc):
                            for rr in range(NR):
                                cr0, crn = g["calls"][rr]
                                if cr0 <= cc < cr0 + crn:
                                    base = g["calls"][(rr // 2) * 2][0]
                                    off = cc - base
                                    half = (rr & 1) * H
                                    return gbufs[rr // 2][
                                        :, off * 2 * H + half : off * 2 * H + half + H
                                    ]
                            raise AssertionError(cc)
                        # group-wide aggregation PSUM + bulk post-agg chain
                        g0 = g["tiles"][0]
                        gw = min(GRP * 128, NPD - g0 * 128)
                        nsl = slice(g0 * 128, g0 * 128 + gw)
                        aps = aggps.tile([H, GRP * 128], f32, tag="aggw")
                        for k, tt in enumerate(g["tiles"]):
                            chs = p.tile_chunks[tt]
                            ksl = slice(k * 128, (k + 1) * 128)
                            if not chs:
                                nc.vector.memset(aps[:, ksl], 0.0)
                                continue
                            for i, cc in enumerate(chs if not no_aggmm else chs[:1]):
                                lc = cc - c0
                                nc.tensor.matmul(
                                    out=aps[:, ksl],
                                    lhsT=val_slice(cc),
                                    rhs=Ab[:, lc * 128 : (lc + 1) * 128],
                                    start=(i == 0),
                                    stop=(i == (0 if no_aggmm else len(chs) - 1)),
                                )
                        hc = pbip.tile([H, GRP * 128], f32, tag="hc")
                        nc.vector.tensor_tensor(
                            out=hc[:, :gw], in0=aps[:, :gw],
                            in1=DGg[:, nsl], op=mybir.AluOpType.add,
                        )
                        nc.vector.tensor_tensor(
                            out=hc[:, :gw], in0=hc[:, :gw], in1=DGd[:, nsl],
                            op=mybir.AluOpType.mult,
                        )
                        nc.scalar.activation(
                            XH[H : 2 * H, nsl], hc[:, :gw], AF.Identity,
                            bias=bc[:, l : l + 1],
                        )
                        mps = mixps.tile([H, GRP * 128], f32, tag="mix")
                        nc.tensor.matmul(
                            out=mps[:, :gw], lhsT=wlt[l][:], rhs=XH[:, nsl],
                            start=True, stop=True,
                        )
                        # residual pre-folded into wlt (host added [I;0])
                        nc.scalar.activation(
                            XH[H : 2 * H, nsl], mps[:, :gw], AF.Identity
                        )

            # ---------------- readout ----------------
            with (
                tc.tile_pool(name="ro", bufs=1) as rop,
                tc.tile_pool(name="rops", bufs=2, space="PSUM") as rops,
            ):
                osb = rop.tile([1, NPD], f32, tag="osb")
                for k0 in range(0, NPD, 512):
                    w = min(512, NPD - k0)
                    nc.scalar.activation(
                        XH[H : 2 * H, k0 : k0 + w],
                        XH[H : 2 * H, k0 : k0 + w], AF.Relu,
                    )
                    pso = rops.tile([1, 512], f32, tag="pso")
                    nc.tensor.matmul(
                        out=pso[:, :w], lhsT=wot[:], rhs=XH[:, k0 : k0 + w],
                        start=True, stop=True,
                    )
                    nc.vector.tensor_copy(osb[:, k0 : k0 + w], pso[:, :w])
                nc.sync.dma_start(out=out_d[:, :], in_=osb[:])

    nc.compile()
    return nc


_CACHE = {}


def prepare(x, edge_index, edge_attr, W1, W2, Wi, bi, w_conv, b_conv, Wl, Wo):
    x = np.asarray(x, dtype=np.float32)
    N, IN_DIM = x.shape
    H = W1.shape[0]
    NL = np.asarray(Wl).shape[0]
    NPD = N // NDEV

    ckey = ("prog", N, edge_index.shape[1], H, IN_DIM, NL)
    if ckey in _CACHE:
        p, nc = _CACHE[ckey]
    else:
        p = preprocess(edge_index, edge_attr, N)
        nc = build_program(p, H, IN_DIM, NL)
        _CACHE[ckey] = (p, nc)

    # block-diagonal (2x) transposed MLP weights for pair-packed P2
    w1tT = np.asarray(W1, np.float32).T  # [8, H]
    w2tT = np.asarray(W2, np.float32).T  # [H, H]
    w1t = np.zeros((16, 2 * H), np.float32)
    w1t[0:8, 0:H] = w1tT
    w1t[8:16, H : 2 * H] = w1tT
    w1t = w1t.astype(nbf16)
    w2t = np.zeros((2 * H, 2 * H), np.float32)
    w2t[0:H, 0:H] = w2tT
    w2t[H : 2 * H, H : 2 * H] = w2tT
    w2t = w2t.astype(nbf16)
    wit = np.ascontiguousarray(np.asarray(Wi, np.float32).T)
    biv = np.asarray(bi, np.float32).reshape(H, 1)
    wcv = np.ascontiguousarray(np.asarray(w_conv, np.float32).T)
    bcv = np.ascontiguousarray(np.asarray(b_conv, np.float32).T)
    wltv = np.ascontiguousarray(np.transpose(np.asarray(Wl, np.float32), (0, 2, 1)))
    # fold the residual h += x_ into the mixing weights: Wl' = Wl + [I; 0]
    wltv = wltv.copy()
    wltv[:, 0:H, :] += np.eye(H, dtype=np.float32)
    wotv = np.ascontiguousarray(np.asarray(Wo, np.float32).T)

    in_maps = []
    for d in range(NDEV):
        in_maps.append(
            {
                "eaT": np.ascontiguousarray(p.eaT[d]),
                "dstf": np.ascontiguousarray(p.dstf[d]),
                "idx16": np.ascontiguousarray(p.idx16[d]),
                "xT": np.ascontiguousarray(x[d * NPD : (d + 1) * NPD, :].T),
                "w1t": w1t, "w2t": w2t, "wit": wit, "bi": biv,
                "wconv": wcv, "bconv": bcv, "wlt": wltv, "wot": wotv,
            }
        )

    return nc, in_maps, NPD


def kernel(x, edge_index, edge_attr, W1, W2, Wi, bi, w_conv, b_conv, Wl, Wo,
           _sim=False):
    nc, in_maps, NPD = prepare(
        x, edge_index, edge_attr, W1, W2, Wi, bi, w_conv, b_conv, Wl, Wo
    )
    if _sim:
        from concourse.bass_interp import MultiCoreSim

        sim = MultiCoreSim(nc, num_cores=NDEV, trace=False)
        cores = list(sim.cores.values())
        for d in range(NDEV):
            for k, v in in_maps[d].items():
                cores[d].tensor(k)[:] = v
        sim.simulate(check_with_hw=False)
        out = np.concatenate(
            [np.array(cores[d].tensor("out")).reshape(NPD, 1) for d in range(NDEV)],
            axis=0,
        )
        return out.astype(np.float32)

    res = run_bass_kernel_spmd(nc, in_maps, list(range(NDEV)))
    out = np.concatenate(
        [res.results[d]["out"].reshape(NPD, 1) for d in range(NDEV)], axis=0
    )
    return out.astype(np.float32)

